# revision 2
# baseline (speedup 1.0000x reference)
"""Causal attention (QKV proj + softmax + PV + ReLU) on 8 trn2 NeuronCores.

v2: fp8 E4M3 DoubleRow matmuls (2 contraction-rows/cycle on the PE) for the
accuracy-tolerant paths, fp16 for the paths whose error hits the output
unaveraged:

  - Q/K projections:  x(e4m3) @ 64*Wq^T(e4m3), DoubleRow pairs over the C
    contraction (4 instrs instead of 8), drained to e4m3 q,k at natural
    scale via one DVE tensor_scalar ((ps + 64b) * 1/64).
  - S = K^T.T @ Q^T:  e4m3 DoubleRow pairs over D (2 instrs instead of 4).
    Softmax-weight perturbation from fp8 logits is ~1.5% and averages out.
  - V projection + PV: j-tile 0 stays fp16 end-to-end — output rows 0..127
    see only v[0:128] with no softmax averaging to suppress v-quantization
    error. j-tiles 1..7 are e4m3 (their error is averaged over >=129
    softmax weights). The whole V path carries a 64x scale (v'=64v,
    rowsum'=64*rowsum via ones=64) so Wv can be prescaled into e4m3's
    normal range; the scale cancels in the O'/rowsum normalization.
  - PV fp8 j-tiles ride DoubleRow in adjacent pairs (1,2),(3,4),(5,6); odd
    leftovers run as plain fp8 matmuls (same 1 cycle/row as fp16 — no loss).

Everything else (host pretranspose, DMA queue split, PE warmup, causal
tile-sparsity, exp-with-bias softmax without max-subtraction, rowsum
piggyback matmul, DVE relu) follows the v1 baseline. Output is f16 (halves
store traffic; adds ~2^-11 relative rounding).
"""

import os
from contextlib import ExitStack

import ml_dtypes
import numpy as np

import concourse.tile as tile
from concourse import bacc, mybir
from concourse import bass_utils

F32 = mybir.dt.float32
F16 = mybir.dt.float16
F8 = mybir.dt.float8e4
AF = mybir.ActivationFunctionType
DR = mybir.MatmulPerfMode.DoubleRow
E4NP = ml_dtypes.float8_e4m3

N_CORES = 8
B = 32
L = 1024
C = 1024  # d_model
D = 512
P = 128
NB = B // N_CORES  # batches per core
CT = C // P  # 8 contraction tiles
DT = D // P  # 4 d tiles
LT = L // P  # 8 l/j/i tiles
SCALE = float(D) ** -0.5
NEG = -30000.0
WS = 64.0  # Q/K weight prescale into e4m3 normal range
# V-path prescale: 32 keeps |32*v| < ~110 with 13-sigma headroom below
# TRN e4m3's 240 saturation point (DVE saturates >240 to +-Inf, unlike
# ml_dtypes' round-to-240, and one Inf in v poisons a whole PV column).
WSV = 32.0


def build_program(nb: int = NB):
    """Build the per-core Bass program for nb batches."""
    nc = bacc.Bacc("TRN2", target_bir_lowering=False, debug=False,
                   num_devices=N_CORES)

    xt8b = nc.dram_tensor("xt8b", [nb, C, L], F8, kind="ExternalInput").ap()
    xt16b = nc.dram_tensor("xt16b", [nb, C, P], F16, kind="ExternalInput").ap()
    wq8 = nc.dram_tensor("wq8", [C, D], F8, kind="ExternalInput").ap()
    wk8 = nc.dram_tensor("wk8", [C, D], F8, kind="ExternalInput").ap()
    wv8 = nc.dram_tensor("wv8", [C, D], F8, kind="ExternalInput").ap()
    wv16 = nc.dram_tensor("wv16", [C, D], F16, kind="ExternalInput").ap()
    bq2 = nc.dram_tensor("bq2", [P, DT], F32, kind="ExternalInput").ap()
    bk2 = nc.dram_tensor("bk2", [P, DT], F32, kind="ExternalInput").ap()
    bvb = nc.dram_tensor("bvb", [P, D], F32, kind="ExternalInput").ap()
    pmt = nc.dram_tensor("pmt", [nb, P, LT], F32, kind="ExternalInput").ap()
    tri16d = nc.dram_tensor("tri16d", [P, P], F16, kind="ExternalInput").ap()
    tri8d = nc.dram_tensor("tri8d", [P, P], F8, kind="ExternalInput").ap()
    out = nc.dram_tensor("out", [nb, L, D], F16, kind="ExternalOutput").ap()

    with tile.TileContext(nc) as tc, ExitStack() as ctx:
        const = ctx.enter_context(tc.tile_pool(name="const", bufs=1))
        x8_pool = ctx.enter_context(tc.tile_pool(name="x8", bufs=3))
        x16_pool = ctx.enter_context(tc.tile_pool(name="x16", bufs=3))
        qk_pool = ctx.enter_context(tc.tile_pool(name="qk", bufs=2))
        v_pool = ctx.enter_context(tc.tile_pool(name="v", bufs=2))
        pt_pool = ctx.enter_context(tc.tile_pool(name="pt", bufs=2))
        o_pool = ctx.enter_context(tc.tile_pool(name="o", bufs=3))
        sm_pool = ctx.enter_context(tc.tile_pool(name="sm", bufs=4))
        pm_pool = ctx.enter_context(tc.tile_pool(name="pm", bufs=2))
        proj_ps = ctx.enter_context(tc.tile_pool(name="pps", bufs=2, space="PSUM"))
        s_ps = ctx.enter_context(tc.tile_pool(name="sps", bufs=2, space="PSUM"))
        o_ps = ctx.enter_context(tc.tile_pool(name="ops", bufs=2, space="PSUM"))
        r_ps = ctx.enter_context(tc.tile_pool(name="rps", bufs=2, space="PSUM"))

        # --- constants, loaded once; scalar HWDGE queue so the sync queue
        # is dedicated to x prefetch ---
        wq_sb = const.tile([P, CT, D], F8)
        nc.scalar.dma_start(wq_sb[:], wq8.rearrange("(t p) d -> p t d", p=P))
        wk_sb = const.tile([P, CT, D], F8)
        wv8_sb = const.tile([P, CT, D], F8)
        wv16_sb = const.tile([P, CT, D], F16)
        bq_sb = const.tile([P, DT], F32)
        nc.scalar.dma_start(bq_sb[:], bq2[:])
        bk_sb = const.tile([P, DT], F32)
        nc.scalar.dma_start(bk_sb[:], bk2[:])
        bv_sb = const.tile([P, D], F32)
        nc.scalar.dma_start(bv_sb[:], bvb[:])
        tri16_sb = const.tile([P, P], F16)
        nc.scalar.dma_start(tri16_sb[:], tri16d[:])
        tri8_sb = const.tile([P, P], F8)
        nc.scalar.dma_start(tri8_sb[:], tri8d[:])
        ones16_sb = const.tile([P, 1], F16)
        nc.vector.memset(ones16_sb[:], WSV)
        ones8_sb = const.tile([P, 2, 1], F8)
        nc.vector.memset(ones8_sb[:], WSV)

        # PE warmup: dummy matmuls with no input deps keep the PE busy while
        # batch-0 inputs stream in (HAM clock-gate to 2.4 GHz).
        warm_sb = const.tile([P, 512], F16)
        nc.vector.memset(warm_sb[:], 0.0)
        for w in range(15):
            wps = proj_ps.tile([P, 512], F32, tag="pp", name=f"warm{w}")
            nc.tensor.matmul(wps[:], warm_sb[:, 0:P], warm_sb[:],
                             start=True, stop=True)

        for b in range(nb):
            # --- X^T tiles: stacked [128, CT, L] e4m3 + [128, CT, 128] f16 ---
            xt8 = x8_pool.tile([P, CT, L], F8, tag="x8", name=f"x8_{b}")
            if b == 0:
                # first batch: l<512 halves of every chunk first so the
                # Q lc=0 groups start earliest
                for ct in range(CT):
                    nc.sync.dma_start(xt8[:, ct, 0:512],
                                      xt8b[b, ct * P:(ct + 1) * P, 0:512])
                for ct in range(CT):
                    nc.sync.dma_start(xt8[:, ct, 512:L],
                                      xt8b[b, ct * P:(ct + 1) * P, 512:L])
            else:
                for ct in range(CT):
                    nc.sync.dma_start(xt8[:, ct, :],
                                      xt8b[b, ct * P:(ct + 1) * P, :])
            xt16 = x16_pool.tile([P, CT, P], F16, tag="x16", name=f"x16_{b}")
            nc.sync.dma_start(xt16[:], xt16b[b].rearrange("(t p) l -> p t l", p=P))
            pm_sb = pm_pool.tile([P, LT], F32, name=f"pm_{b}")
            nc.sync.dma_start(pm_sb[:], pmt[b])
            if b == 0:
                # deferred behind batch-0 x in the sync ring: these don't
                # steal HBM bandwidth from the startup-critical prefetch
                nc.sync.dma_start(wk_sb[:],
                                  wk8.rearrange("(t p) d -> p t d", p=P))
                nc.sync.dma_start(wv16_sb[:],
                                  wv16.rearrange("(t p) d -> p t d", p=P))
                nc.sync.dma_start(wv8_sb[:],
                                  wv8.rearrange("(t p) d -> p t d", p=P))

            # --- Q^T, K^T: [128d, DT, L] f16, natural scale (fp8 proj, but
            # fp16 carrier into S so no requantization error) ---
            qt = qk_pool.tile([P, DT, L], F16, tag="q16", name=f"q16_{b}")
            kt = qk_pool.tile([P, DT, L], F16, tag="k16", name=f"k16_{b}")
            for name, w_sb, b_sb, dst in (("q", wq_sb, bq_sb, qt),
                                          ("k", wk_sb, bk_sb, kt)):
                if b == 0 and name == "q":
                    # lc-outer so all lc=0 groups run on the early halves
                    order = [(dt, lc) for lc in range(L // 512)
                             for dt in range(DT)]
                else:
                    order = [(dt, lc) for dt in range(DT)
                             for lc in range(L // 512)]
                for dt, lc in order:
                    ps = proj_ps.tile([P, 512], F32, tag="pp",
                                      name=f"{name}ps{dt}_{lc}_{b}")
                    for c in range(CT // 2):
                        nc.tensor.matmul(
                            ps[:],
                            w_sb[:, 2 * c:2 * c + 2, dt * P:(dt + 1) * P],
                            xt8[:, 2 * c:2 * c + 2, lc * 512:(lc + 1) * 512],
                            start=(c == 0), stop=(c == CT // 2 - 1),
                            perf_mode=DR)
                    # (ps + 64b) * 1/64 -> e4m3 at natural scale
                    nc.vector.tensor_scalar(
                        dst[:, dt, lc * 512:(lc + 1) * 512], ps[:],
                        b_sb[:, dt:dt + 1], 1.0 / WS,
                        mybir.AluOpType.add, mybir.AluOpType.mult)

            # --- V' = 64*V: j-tile 0 fp16, tiles 1..7 e4m3 ---
            v16 = v_pool.tile([P, D], F16, tag="v16", name=f"v16_{b}")
            v8 = v_pool.tile([P, LT - 1, D], F8, tag="v8", name=f"v8_{b}")
            ps = proj_ps.tile([P, D], F32, tag="pp", name=f"v16ps_{b}")
            for ct in range(CT):
                nc.tensor.matmul(ps[:], xt16[:, ct, :], wv16_sb[:, ct, :],
                                 start=(ct == 0), stop=(ct == CT - 1))
            nc.vector.tensor_add(v16[:], ps[:], bv_sb[:])
            for lt in range(1, LT):
                ps = proj_ps.tile([P, D], F32, tag="pp", name=f"v8ps{lt}_{b}")
                for c in range(CT // 2):
                    nc.tensor.matmul(
                        ps[:],
                        xt8[:, 2 * c:2 * c + 2, lt * P:(lt + 1) * P],
                        wv8_sb[:, 2 * c:2 * c + 2, :],
                        start=(c == 0), stop=(c == CT // 2 - 1),
                        perf_mode=DR)
                nc.vector.tensor_add(v8[:, lt - 1, :], ps[:], bv_sb[:])

            # --- S^T tiles + exp -> P^T (causal: only i >= j0 computed) ---
            pt16 = pt_pool.tile([P, L], F16, tag="p16", name=f"p16_{b}")
            pt8 = pt_pool.tile([P, LT, L], F8, tag="p8", name=f"p8_{b}")
            for jb in range(LT):
                j0 = jb * P
                i0 = j0
                while i0 < L:
                    n = min((i0 // 512 + 1) * 512, L) - i0
                    ps = s_ps.tile([P, n], F32, tag="sp", name=f"sps{jb}_{i0}_{b}")
                    for a in range(DT):
                        nc.tensor.matmul(
                            ps[:],
                            kt[:, a, j0:j0 + P],
                            qt[:, a, i0:i0 + n],
                            start=(a == 0), stop=(a == DT - 1))
                    tgt = pt16[:, i0:i0 + n] if jb == 0 else pt8[:, jb, i0:i0 + n]
                    nc.scalar.activation(tgt, ps[:], AF.Exp,
                                         bias=pm_sb[:, jb:jb + 1], scale=SCALE)
                    i0 += n
                # mask the diagonal tile: keep j<=i (upper-right triangle)
                if jb == 0:
                    nc.vector.tensor_mul(pt16[:, 0:P], pt16[:, 0:P], tri16_sb[:])
                else:
                    nc.vector.tensor_mul(pt8[:, jb, j0:j0 + P],
                                         pt8[:, jb, j0:j0 + P], tri8_sb[:])

            # --- O' = P^T.T @ V', rowsum' = P^T.T @ 64*ones; norm+relu ---
            for ib in range(LT):
                i0 = ib * P
                ops = o_ps.tile([P, D], F32, tag="op", name=f"ops{ib}_{b}")
                rps = r_ps.tile([P, 1], F32, tag="rp", name=f"rps{ib}_{b}")
                # fp16 j-tile 0
                nc.tensor.matmul(ops[:], pt16[:, i0:i0 + P], v16[:],
                                 start=True, stop=(ib == 0))
                nc.tensor.matmul(rps[:], pt16[:, i0:i0 + P], ones16_sb[:],
                                 start=True, stop=(ib == 0))
                # fp8 j-tiles 1..ib: DoubleRow pairs + odd single
                js = 1
                while js <= ib:
                    last = js + 2 > ib  # this step consumes through ib
                    if js + 1 <= ib:
                        nc.tensor.matmul(ops[:], pt8[:, js:js + 2, i0:i0 + P],
                                         v8[:, js - 1:js + 1, :],
                                         start=False, stop=last, perf_mode=DR)
                        nc.tensor.matmul(rps[:], pt8[:, js:js + 2, i0:i0 + P],
                                         ones8_sb[:, 0:2, :],
                                         start=False, stop=last, perf_mode=DR)
                        js += 2
                    else:
                        nc.tensor.matmul(ops[:], pt8[:, js, i0:i0 + P],
                                         v8[:, js - 1, :],
                                         start=False, stop=True)
                        nc.tensor.matmul(rps[:], pt8[:, js, i0:i0 + P],
                                         ones8_sb[:, 0, 0:1],
                                         start=False, stop=True)
                        js += 1
                rec = sm_pool.tile([P, 1], F32, tag="rec", name=f"rec{ib}_{b}")
                nc.vector.reciprocal(rec[:], rps[:])
                o_sb = o_pool.tile([P, D], F16, tag="ot", name=f"o{ib}_{b}")
                # relu(O'/rowsum') on DVE: (in0 * rec) max 0
                nc.vector.tensor_scalar(o_sb[:], ops[:], rec[:], 0.0,
                                        mybir.AluOpType.mult,
                                        mybir.AluOpType.max)
                # SWDGE so stores never head-of-line-block the x prefetch;
                # last batch has no prefetch left -> faster HWDGE
                if b == nb - 1:
                    nc.sync.dma_start(out[b, i0:i0 + P, :], o_sb[:])
                else:
                    nc.gpsimd.dma_start(out[b, i0:i0 + P, :], o_sb[:])

    nc.compile()
    return nc


def _prep_host(x, Wq, bq, Wk, bk, Wv, bv, mask):
    xT = x.transpose(0, 2, 1)  # [B, C, L] view
    xb8 = np.ascontiguousarray(xT).astype(E4NP)
    xb16 = np.ascontiguousarray(xT[:, :, 0:P]).astype(np.float16)
    wq8T = (Wq.T * WS).astype(E4NP)  # [C, D]
    wk8T = (Wk.T * WS).astype(E4NP)
    wv8T = (Wv.T * WSV).astype(E4NP)
    wv16T = (Wv.T * WSV).astype(np.float16)  # V path carries the 32x scale
    bq2 = np.ascontiguousarray(
        (WS * bq).astype(np.float32).reshape(DT, P).T)  # [P, DT]
    bk2 = np.ascontiguousarray((WS * bk).astype(np.float32).reshape(DT, P).T)
    bvb = np.ascontiguousarray(
        np.broadcast_to((WSV * bv).astype(np.float32), (P, D)))  # [P, D]
    pm = np.where(mask[:, 0, :] != 0, 0.0, NEG).astype(np.float32)  # [B, L]
    pmt = np.ascontiguousarray(
        pm.reshape(B, LT, P).transpose(0, 2, 1))  # [B, P, LT]
    tri = (np.arange(P)[:, None] <= np.arange(P)[None, :])
    return (xb8, xb16, wq8T, wk8T, wv8T, wv16T, bq2, bk2, bvb, pmt,
            tri.astype(np.float16), tri.astype(E4NP))


_NC_CACHE = {}


def kernel(x, Wq, bq, Wk, bk, Wv, bv, mask):
    x = np.asarray(x)
    Wq, bq = np.asarray(Wq), np.asarray(bq)
    Wk, bk = np.asarray(Wk), np.asarray(bk)
    Wv, bv = np.asarray(Wv), np.asarray(bv)
    mask = np.asarray(mask)

    (xb8, xb16, wq8T, wk8T, wv8T, wv16T, bq2, bk2, bvb, pmt,
     tri16, tri8) = _prep_host(x, Wq, bq, Wk, bk, Wv, bv, mask)

    if "nc" not in _NC_CACHE:
        _NC_CACHE["nc"] = build_program(NB)
    nc = _NC_CACHE["nc"]

    in_maps = []
    for c in range(N_CORES):
        s = slice(c * NB, (c + 1) * NB)
        in_maps.append({
            "xt8b": np.ascontiguousarray(xb8[s]),
            "xt16b": np.ascontiguousarray(xb16[s]),
            "wq8": wq8T, "wk8": wk8T, "wv8": wv8T, "wv16": wv16T,
            "bq2": bq2, "bk2": bk2, "bvb": bvb,
            "pmt": np.ascontiguousarray(pmt[s]),
            "tri16d": tri16, "tri8d": tri8,
        })

    res = bass_utils.run_bass_kernel_spmd(
        nc, in_maps, core_ids=list(range(N_CORES)),
        trace=bool(int(os.environ.get("KERNEL_TRACE", "0"))),
    )
    if os.environ.get("KERNEL_RESULT_HOOK"):
        _NC_CACHE["last_result"] = res

    return np.concatenate(
        [res.results[c]["out"].astype(np.float32) for c in range(N_CORES)],
        axis=0)


# revision 3
# speedup vs baseline: 1.0216x; 1.0216x over previous
"""Causal attention (QKV proj + softmax + PV + ReLU) on 8 trn2 NeuronCores.

v4: fp8 E4M3 DoubleRow matmuls (2 contraction rows/cycle on the PE)
everywhere softmax averaging suppresses quantization noise; fp16 only for
the causal corner whose rows see no averaging:

  - Q/K projections: x(e4m3) @ 64*W^T(e4m3) DoubleRow over C (4 instrs
    instead of 8). Drained twice: columns l<128 to fp16 carriers (for the
    corner S) and the rest straight to e4m3 carriers (for the DoubleRow S).
  - S = K^T.T @ Q^T: output rows i<128 (one n=128 chunk of j-tile 0) in
    fp16 from the fp16 carriers — these rows average <=128 softmax terms
    and dominate the max-error. Everything else (i>=128) in e4m3 DoubleRow
    pairs over D: logit noise there is suppressed by >=1/sqrt(129) weight
    averaging.
  - V projection: j-tile 0 computed in fp16 (x16 @ 32*Wv^T f16) and drained
    both to a fp16 tile (for output rows <128) and to e4m3 (for rows >=128);
    j-tiles 1..7 e4m3 DoubleRow. The V path carries a 32x scale — 32 (not
    64) keeps |32*v| ~13 sigma below TRN e4m3's 240 saturation point (the
    DVE saturates >240 to +-Inf, and one Inf poisons a whole PV column).
    The scale cancels in the O'/rowsum normalization (ones=32).
  - PV: output tile ib=0 fp16 (P16 @ V16); ib>=1 all-fp8 with DoubleRow
    j-tile pairs (0,1),(2,3),(4,5) + odd single; rowsum piggybacks on the
    same stationary P^T at N=1.

relu(O'/rowsum) moved to the ACT engine (activation Relu with scale=1/rs
per-partition AP) to keep the DVE queue short — DVE carries the fp8 drains.
Host prep, DMA queue split, PE warmup, exp-with-bias softmax and causal
tile sparsity follow the v1 baseline. Output f16.
"""

import os
from contextlib import ExitStack

import ml_dtypes
import numpy as np

import concourse.tile as tile
from concourse import bacc, mybir
from concourse import bass_utils

F32 = mybir.dt.float32
F16 = mybir.dt.float16
F8 = mybir.dt.float8e4
AF = mybir.ActivationFunctionType
DR = mybir.MatmulPerfMode.DoubleRow
E4NP = ml_dtypes.float8_e4m3

N_CORES = 8
B = 32
L = 1024
C = 1024  # d_model
D = 512
P = 128
NB = B // N_CORES  # batches per core
CT = C // P  # 8 contraction tiles
DT = D // P  # 4 d tiles
LT = L // P  # 8 l/j/i tiles
SCALE = float(D) ** -0.5
NEG = -30000.0
WS = 64.0   # Q/K weight prescale into e4m3 normal range
WSV = 32.0  # V-path prescale (see module docstring)


def build_program(nb: int = NB):
    """Build the per-core Bass program for nb batches."""
    nc = bacc.Bacc("TRN2", target_bir_lowering=False, debug=False,
                   num_devices=N_CORES)

    xt8b = nc.dram_tensor("xt8b", [nb, C, L], F8, kind="ExternalInput").ap()
    xt16b = nc.dram_tensor("xt16b", [nb, C, P], F16, kind="ExternalInput").ap()
    wq8 = nc.dram_tensor("wq8", [C, D], F8, kind="ExternalInput").ap()
    wk8 = nc.dram_tensor("wk8", [C, D], F8, kind="ExternalInput").ap()
    wv8 = nc.dram_tensor("wv8", [C, D], F8, kind="ExternalInput").ap()
    wv16 = nc.dram_tensor("wv16", [C, D], F16, kind="ExternalInput").ap()
    bq2 = nc.dram_tensor("bq2", [P, DT], F32, kind="ExternalInput").ap()
    bk2 = nc.dram_tensor("bk2", [P, DT], F32, kind="ExternalInput").ap()
    bvb = nc.dram_tensor("bvb", [P, D], F32, kind="ExternalInput").ap()
    pmt = nc.dram_tensor("pmt", [nb, P, LT], F32, kind="ExternalInput").ap()
    tri16d = nc.dram_tensor("tri16d", [P, P], F16, kind="ExternalInput").ap()
    tri8d = nc.dram_tensor("tri8d", [P, P], F8, kind="ExternalInput").ap()
    out = nc.dram_tensor("out", [nb, L, D], F16, kind="ExternalOutput").ap()

    with tile.TileContext(nc) as tc, ExitStack() as ctx:
        const = ctx.enter_context(tc.tile_pool(name="const", bufs=1))
        x8_pool = ctx.enter_context(tc.tile_pool(name="x8", bufs=3))
        x16_pool = ctx.enter_context(tc.tile_pool(name="x16", bufs=3))
        qk_pool = ctx.enter_context(tc.tile_pool(name="qk", bufs=2))
        v_pool = ctx.enter_context(tc.tile_pool(name="v", bufs=2))
        pt_pool = ctx.enter_context(tc.tile_pool(name="pt", bufs=2))
        o_pool = ctx.enter_context(tc.tile_pool(name="o", bufs=3))
        sm_pool = ctx.enter_context(tc.tile_pool(name="sm", bufs=4))
        pm_pool = ctx.enter_context(tc.tile_pool(name="pm", bufs=2))
        proj_ps = ctx.enter_context(tc.tile_pool(name="pps", bufs=2, space="PSUM"))
        s_ps = ctx.enter_context(tc.tile_pool(name="sps", bufs=2, space="PSUM"))
        o_ps = ctx.enter_context(tc.tile_pool(name="ops", bufs=2, space="PSUM"))
        r_ps = ctx.enter_context(tc.tile_pool(name="rps", bufs=2, space="PSUM"))

        # --- constants, loaded once; scalar HWDGE queue so the sync queue
        # is dedicated to x prefetch ---
        wq_sb = const.tile([P, CT, D], F8)
        nc.scalar.dma_start(wq_sb[:], wq8.rearrange("(t p) d -> p t d", p=P))
        wk_sb = const.tile([P, CT, D], F8)
        wv8_sb = const.tile([P, CT, D], F8)
        wv16_sb = const.tile([P, CT, D], F16)
        bq_sb = const.tile([P, DT], F32)
        nc.scalar.dma_start(bq_sb[:], bq2[:])
        bk_sb = const.tile([P, DT], F32)
        nc.scalar.dma_start(bk_sb[:], bk2[:])
        bv_sb = const.tile([P, D], F32)
        nc.scalar.dma_start(bv_sb[:], bvb[:])
        tri16_sb = const.tile([P, P], F16)
        nc.scalar.dma_start(tri16_sb[:], tri16d[:])
        tri8_sb = const.tile([P, P], F8)
        nc.scalar.dma_start(tri8_sb[:], tri8d[:])
        ones16_sb = const.tile([P, 1], F16)
        nc.vector.memset(ones16_sb[:], WSV)
        ones8_sb = const.tile([P, 2, 1], F8)
        nc.vector.memset(ones8_sb[:], WSV)

        # PE warmup: dummy matmuls with no input deps keep the PE busy while
        # batch-0 inputs stream in (HAM clock-gate to 2.4 GHz).
        warm_sb = const.tile([P, 512], F16)
        nc.vector.memset(warm_sb[:], 0.0)
        for w in range(15):
            wps = proj_ps.tile([P, 512], F32, tag="pp", name=f"warm{w}")
            nc.tensor.matmul(wps[:], warm_sb[:, 0:P], warm_sb[:],
                             start=True, stop=True)

        for b in range(nb):
            # --- X^T tiles: stacked [128, CT, L] e4m3 + [128, CT, 128] f16 ---
            xt8 = x8_pool.tile([P, CT, L], F8, tag="x8", name=f"x8_{b}")
            if b == 0:
                # first batch: l<512 halves of every chunk first so the
                # Q lc=0 groups start earliest
                for ct in range(CT):
                    nc.sync.dma_start(xt8[:, ct, 0:512],
                                      xt8b[b, ct * P:(ct + 1) * P, 0:512])
                for ct in range(CT):
                    nc.sync.dma_start(xt8[:, ct, 512:L],
                                      xt8b[b, ct * P:(ct + 1) * P, 512:L])
            else:
                for ct in range(CT):
                    nc.sync.dma_start(xt8[:, ct, :],
                                      xt8b[b, ct * P:(ct + 1) * P, :])
            xt16 = x16_pool.tile([P, CT, P], F16, tag="x16", name=f"x16_{b}")
            nc.sync.dma_start(xt16[:], xt16b[b].rearrange("(t p) l -> p t l", p=P))
            pm_sb = pm_pool.tile([P, LT], F32, name=f"pm_{b}")
            nc.sync.dma_start(pm_sb[:], pmt[b])
            if b == 0:
                # deferred behind batch-0 x in the sync ring: these don't
                # steal HBM bandwidth from the startup-critical prefetch
                nc.sync.dma_start(wk_sb[:],
                                  wk8.rearrange("(t p) d -> p t d", p=P))
                nc.sync.dma_start(wv16_sb[:],
                                  wv16.rearrange("(t p) d -> p t d", p=P))
                nc.sync.dma_start(wv8_sb[:],
                                  wv8.rearrange("(t p) d -> p t d", p=P))

            # --- Q^T, K^T: fp8 DoubleRow proj; dual-drained carriers:
            # [128d, DT, 128] f16 (corner) + [128d, DT, L] e4m3 (rest) ---
            qt16 = qk_pool.tile([P, DT, P], F16, tag="q16", name=f"q16_{b}")
            qt8 = qk_pool.tile([P, DT, L], F8, tag="q8", name=f"q8_{b}")
            kt16 = qk_pool.tile([P, DT, P], F16, tag="k16", name=f"k16_{b}")
            kt8 = qk_pool.tile([P, DT, L], F8, tag="k8", name=f"k8_{b}")
            for name, w_sb, b_sb, d16, d8 in (
                    ("q", wq_sb, bq_sb, qt16, qt8),
                    ("k", wk_sb, bk_sb, kt16, kt8)):
                if b == 0 and name == "q":
                    # lc-outer so all lc=0 groups run on the early halves
                    order = [(dt, lc) for lc in range(L // 512)
                             for dt in range(DT)]
                else:
                    order = [(dt, lc) for dt in range(DT)
                             for lc in range(L // 512)]
                for dt, lc in order:
                    ps = proj_ps.tile([P, 512], F32, tag="pp",
                                      name=f"{name}ps{dt}_{lc}_{b}")
                    for c in range(CT // 2):
                        nc.tensor.matmul(
                            ps[:],
                            w_sb[:, 2 * c:2 * c + 2, dt * P:(dt + 1) * P],
                            xt8[:, 2 * c:2 * c + 2, lc * 512:(lc + 1) * 512],
                            start=(c == 0), stop=(c == CT // 2 - 1),
                            perf_mode=DR)
                    # (ps + 64b) * 1/64 -> natural-scale carriers
                    if lc == 0:
                        nc.vector.tensor_scalar(
                            d16[:, dt, :], ps[:, 0:P],
                            b_sb[:, dt:dt + 1], 1.0 / WS,
                            mybir.AluOpType.add, mybir.AluOpType.mult)
                        if name == "q":
                            nc.vector.tensor_scalar(
                                d8[:, dt, P:512], ps[:, P:512],
                                b_sb[:, dt:dt + 1], 1.0 / WS,
                                mybir.AluOpType.add, mybir.AluOpType.mult)
                        else:
                            # k8 j<128 is DR-stationary for i>=128 chunks
                            nc.vector.tensor_scalar(
                                d8[:, dt, 0:512], ps[:],
                                b_sb[:, dt:dt + 1], 1.0 / WS,
                                mybir.AluOpType.add, mybir.AluOpType.mult)
                    else:
                        nc.vector.tensor_scalar(
                            d8[:, dt, 512:L], ps[:],
                            b_sb[:, dt:dt + 1], 1.0 / WS,
                            mybir.AluOpType.add, mybir.AluOpType.mult)

            # --- V' = 32*V: j-tile 0 fp16 (dual-drained f16+f8), 1..7 e4m3 ---
            v16 = v_pool.tile([P, D], F16, tag="v16", name=f"v16_{b}")
            v8 = v_pool.tile([P, LT, D], F8, tag="v8", name=f"v8_{b}")
            ps = proj_ps.tile([P, D], F32, tag="pp", name=f"v16ps_{b}")
            for ct in range(CT):
                nc.tensor.matmul(ps[:], xt16[:, ct, :], wv16_sb[:, ct, :],
                                 start=(ct == 0), stop=(ct == CT - 1))
            nc.vector.tensor_add(v16[:], ps[:], bv_sb[:])
            nc.vector.tensor_add(v8[:, 0, :], ps[:], bv_sb[:])
            for lt in range(1, LT):
                ps = proj_ps.tile([P, D], F32, tag="pp", name=f"v8ps{lt}_{b}")
                for c in range(CT // 2):
                    nc.tensor.matmul(
                        ps[:],
                        xt8[:, 2 * c:2 * c + 2, lt * P:(lt + 1) * P],
                        wv8_sb[:, 2 * c:2 * c + 2, :],
                        start=(c == 0), stop=(c == CT // 2 - 1),
                        perf_mode=DR)
                nc.vector.tensor_add(v8[:, lt, :], ps[:], bv_sb[:])

            # --- S^T tiles + exp -> P^T: fp16 corner (i<128), fp8 DR rest ---
            pt16 = pt_pool.tile([P, P], F16, tag="p16", name=f"p16_{b}")
            pt8 = pt_pool.tile([P, LT, L], F8, tag="p8", name=f"p8_{b}")
            for jb in range(LT):
                j0 = jb * P
                if jb == 0:
                    # fp16 corner: rows i<128 see no softmax averaging
                    ps = s_ps.tile([P, P], F32, tag="sp", name=f"spc_{b}")
                    for a in range(DT):
                        nc.tensor.matmul(ps[:], kt16[:, a, :], qt16[:, a, :],
                                         start=(a == 0), stop=(a == DT - 1))
                    nc.scalar.activation(pt16[:], ps[:], AF.Exp,
                                         bias=pm_sb[:, 0:1], scale=SCALE)
                    nc.vector.tensor_mul(pt16[:], pt16[:], tri16_sb[:])
                    i0 = P
                else:
                    i0 = j0
                while i0 < L:
                    n = min((i0 // 512 + 1) * 512, L) - i0
                    ps = s_ps.tile([P, n], F32, tag="sp", name=f"sps{jb}_{i0}_{b}")
                    for a in range(DT // 2):
                        nc.tensor.matmul(
                            ps[:],
                            kt8[:, 2 * a:2 * a + 2, j0:j0 + P],
                            qt8[:, 2 * a:2 * a + 2, i0:i0 + n],
                            start=(a == 0), stop=(a == DT // 2 - 1),
                            perf_mode=DR)
                    nc.scalar.activation(pt8[:, jb, i0:i0 + n], ps[:], AF.Exp,
                                         bias=pm_sb[:, jb:jb + 1], scale=SCALE)
                    i0 += n
                if jb > 0:
                    # mask the diagonal tile: keep j<=i
                    nc.vector.tensor_mul(pt8[:, jb, j0:j0 + P],
                                         pt8[:, jb, j0:j0 + P], tri8_sb[:])

            # --- O' = P^T.T @ V', rowsum' = P^T.T @ 32*ones; norm+relu ---
            for ib in range(LT):
                i0 = ib * P
                ops = o_ps.tile([P, D], F32, tag="op", name=f"ops{ib}_{b}")
                rps = r_ps.tile([P, 1], F32, tag="rp", name=f"rps{ib}_{b}")
                if ib == 0:
                    nc.tensor.matmul(ops[:], pt16[:], v16[:],
                                     start=True, stop=True)
                    nc.tensor.matmul(rps[:], pt16[:], ones16_sb[:],
                                     start=True, stop=True)
                else:
                    js = 0
                    while js <= ib:
                        first = js == 0
                        if js + 1 <= ib:
                            last = js + 2 > ib
                            nc.tensor.matmul(ops[:], pt8[:, js:js + 2, i0:i0 + P],
                                             v8[:, js:js + 2, :],
                                             start=first, stop=last,
                                             perf_mode=DR)
                            nc.tensor.matmul(rps[:], pt8[:, js:js + 2, i0:i0 + P],
                                             ones8_sb[:, 0:2, :],
                                             start=first, stop=last,
                                             perf_mode=DR)
                            js += 2
                        else:
                            nc.tensor.matmul(ops[:], pt8[:, js, i0:i0 + P],
                                             v8[:, js, :],
                                             start=first, stop=True)
                            nc.tensor.matmul(rps[:], pt8[:, js, i0:i0 + P],
                                             ones8_sb[:, 0, 0:1],
                                             start=first, stop=True)
                            js += 1
                rec = sm_pool.tile([P, 1], F32, tag="rec", name=f"rec{ib}_{b}")
                nc.vector.reciprocal(rec[:], rps[:])
                o_sb = o_pool.tile([P, D], F16, tag="ot", name=f"o{ib}_{b}")
                # relu(O'/rowsum') on ACT: Relu(ops * rec + 0) with rec as a
                # per-partition scale AP — keeps the DVE queue short
                nc.scalar.activation(o_sb[:], ops[:], AF.Relu,
                                     bias=0.0, scale=rec[:])
                # SWDGE so stores never head-of-line-block the x prefetch;
                # last batch has no prefetch left -> faster HWDGE
                if b == nb - 1:
                    nc.sync.dma_start(out[b, i0:i0 + P, :], o_sb[:])
                else:
                    nc.gpsimd.dma_start(out[b, i0:i0 + P, :], o_sb[:])

    nc.compile()
    return nc


def _prep_host(x, Wq, bq, Wk, bk, Wv, bv, mask):
    xT = x.transpose(0, 2, 1)  # [B, C, L] view
    xb8 = np.ascontiguousarray(xT).astype(E4NP)
    xb16 = np.ascontiguousarray(xT[:, :, 0:P]).astype(np.float16)
    wq8T = (Wq.T * WS).astype(E4NP)  # [C, D]
    wk8T = (Wk.T * WS).astype(E4NP)
    wv8T = (Wv.T * WSV).astype(E4NP)
    wv16T = (Wv.T * WSV).astype(np.float16)
    bq2 = np.ascontiguousarray(
        (WS * bq).astype(np.float32).reshape(DT, P).T)  # [P, DT]
    bk2 = np.ascontiguousarray((WS * bk).astype(np.float32).reshape(DT, P).T)
    bvb = np.ascontiguousarray(
        np.broadcast_to((WSV * bv).astype(np.float32), (P, D)))  # [P, D]
    pm = np.where(mask[:, 0, :] != 0, 0.0, NEG).astype(np.float32)  # [B, L]
    pmt = np.ascontiguousarray(
        pm.reshape(B, LT, P).transpose(0, 2, 1))  # [B, P, LT]
    tri = (np.arange(P)[:, None] <= np.arange(P)[None, :])
    return (xb8, xb16, wq8T, wk8T, wv8T, wv16T, bq2, bk2, bvb, pmt,
            tri.astype(np.float16), tri.astype(E4NP))


_NC_CACHE = {}


def kernel(x, Wq, bq, Wk, bk, Wv, bv, mask):
    x = np.asarray(x)
    Wq, bq = np.asarray(Wq), np.asarray(bq)
    Wk, bk = np.asarray(Wk), np.asarray(bk)
    Wv, bv = np.asarray(Wv), np.asarray(bv)
    mask = np.asarray(mask)

    (xb8, xb16, wq8T, wk8T, wv8T, wv16T, bq2, bk2, bvb, pmt,
     tri16, tri8) = _prep_host(x, Wq, bq, Wk, bk, Wv, bv, mask)

    if "nc" not in _NC_CACHE:
        _NC_CACHE["nc"] = build_program(NB)
    nc = _NC_CACHE["nc"]

    in_maps = []
    for c in range(N_CORES):
        s = slice(c * NB, (c + 1) * NB)
        in_maps.append({
            "xt8b": np.ascontiguousarray(xb8[s]),
            "xt16b": np.ascontiguousarray(xb16[s]),
            "wq8": wq8T, "wk8": wk8T, "wv8": wv8T, "wv16": wv16T,
            "bq2": bq2, "bk2": bk2, "bvb": bvb,
            "pmt": np.ascontiguousarray(pmt[s]),
            "tri16d": tri16, "tri8d": tri8,
        })

    res = bass_utils.run_bass_kernel_spmd(
        nc, in_maps, core_ids=list(range(N_CORES)),
        trace=bool(int(os.environ.get("KERNEL_TRACE", "0"))),
    )
    if os.environ.get("KERNEL_RESULT_HOOK"):
        _NC_CACHE["last_result"] = res

    return np.concatenate(
        [res.results[c]["out"].astype(np.float32) for c in range(N_CORES)],
        axis=0)


# revision 4
# speedup vs baseline: 1.0263x; 1.0046x over previous
"""Causal attention (QKV proj + softmax + PV + ReLU) on 8 trn2 NeuronCores.

v4: fp8 E4M3 DoubleRow matmuls (2 contraction rows/cycle on the PE)
everywhere softmax averaging suppresses quantization noise; fp16 only for
the causal corner whose rows see no averaging:

  - Q/K projections: x(e4m3) @ 64*W^T(e4m3) DoubleRow over C (4 instrs
    instead of 8). Drained twice: columns l<128 to fp16 carriers (for the
    corner S) and the rest straight to e4m3 carriers (for the DoubleRow S).
  - S = K^T.T @ Q^T: output rows i<128 (one n=128 chunk of j-tile 0) in
    fp16 from the fp16 carriers — these rows average <=128 softmax terms
    and dominate the max-error. Everything else (i>=128) in e4m3 DoubleRow
    pairs over D: logit noise there is suppressed by >=1/sqrt(129) weight
    averaging.
  - V projection: j-tile 0 computed in fp16 (x16 @ 32*Wv^T f16) and drained
    both to a fp16 tile (for output rows <128) and to e4m3 (for rows >=128);
    j-tiles 1..7 e4m3 DoubleRow. The V path carries a 32x scale — 32 (not
    64) keeps |32*v| ~13 sigma below TRN e4m3's 240 saturation point (the
    DVE saturates >240 to +-Inf, and one Inf poisons a whole PV column).
    The scale cancels in the O'/rowsum normalization (ones=32).
  - PV: output tile ib=0 fp16 (P16 @ V16); ib>=1 all-fp8 with DoubleRow
    j-tile pairs (0,1),(2,3),(4,5) + odd single; rowsum piggybacks on the
    same stationary P^T at N=1.

relu(O'/rowsum) moved to the ACT engine (activation Relu with scale=1/rs
per-partition AP) to keep the DVE queue short — DVE carries the fp8 drains.
Host prep, DMA queue split, PE warmup, exp-with-bias softmax and causal
tile sparsity follow the v1 baseline. Output f16.
"""

import os
from contextlib import ExitStack

import ml_dtypes
import numpy as np

import concourse.tile as tile
from concourse import bacc, mybir
from concourse import bass_utils

F32 = mybir.dt.float32
F16 = mybir.dt.float16
F8 = mybir.dt.float8e4
AF = mybir.ActivationFunctionType
DR = mybir.MatmulPerfMode.DoubleRow
E4NP = ml_dtypes.float8_e4m3

N_CORES = 8
B = 32
L = 1024
C = 1024  # d_model
D = 512
P = 128
NB = B // N_CORES  # batches per core
CT = C // P  # 8 contraction tiles
DT = D // P  # 4 d tiles
LT = L // P  # 8 l/j/i tiles
SCALE = float(D) ** -0.5
NEG = -30000.0
WS = 64.0   # Q/K weight prescale into e4m3 normal range
WSV = 32.0  # V-path prescale (see module docstring)


def build_program(nb: int = NB):
    """Build the per-core Bass program for nb batches."""
    nc = bacc.Bacc("TRN2", target_bir_lowering=False, debug=False,
                   num_devices=N_CORES)

    xt8b = nc.dram_tensor("xt8b", [nb, C, L], F8, kind="ExternalInput").ap()
    xt16b = nc.dram_tensor("xt16b", [nb, C, P], F16, kind="ExternalInput").ap()
    wq8 = nc.dram_tensor("wq8", [C, D], F8, kind="ExternalInput").ap()
    wk8 = nc.dram_tensor("wk8", [C, D], F8, kind="ExternalInput").ap()
    wv8 = nc.dram_tensor("wv8", [C, D], F8, kind="ExternalInput").ap()
    wv16 = nc.dram_tensor("wv16", [C, D], F16, kind="ExternalInput").ap()
    bq2 = nc.dram_tensor("bq2", [P, DT], F32, kind="ExternalInput").ap()
    bk2 = nc.dram_tensor("bk2", [P, DT], F32, kind="ExternalInput").ap()
    bvb = nc.dram_tensor("bvb", [P, D], F32, kind="ExternalInput").ap()
    pmt = nc.dram_tensor("pmt", [nb, P, LT], F32, kind="ExternalInput").ap()
    tri16d = nc.dram_tensor("tri16d", [P, P], F16, kind="ExternalInput").ap()
    tri8d = nc.dram_tensor("tri8d", [P, P], F8, kind="ExternalInput").ap()
    out = nc.dram_tensor("out", [nb, L, D], F16, kind="ExternalOutput").ap()

    with tile.TileContext(nc) as tc, ExitStack() as ctx:
        const = ctx.enter_context(tc.tile_pool(name="const", bufs=1))
        x8_pool = ctx.enter_context(tc.tile_pool(name="x8", bufs=3))
        x16_pool = ctx.enter_context(tc.tile_pool(name="x16", bufs=3))
        qk_pool = ctx.enter_context(tc.tile_pool(name="qk", bufs=2))
        v_pool = ctx.enter_context(tc.tile_pool(name="v", bufs=2))
        pt_pool = ctx.enter_context(tc.tile_pool(name="pt", bufs=2))
        o_pool = ctx.enter_context(tc.tile_pool(name="o", bufs=3))
        sm_pool = ctx.enter_context(tc.tile_pool(name="sm", bufs=4))
        pm_pool = ctx.enter_context(tc.tile_pool(name="pm", bufs=2))
        proj_ps = ctx.enter_context(tc.tile_pool(name="pps", bufs=2, space="PSUM"))
        s_ps = ctx.enter_context(tc.tile_pool(name="sps", bufs=2, space="PSUM"))
        o_ps = ctx.enter_context(tc.tile_pool(name="ops", bufs=3, space="PSUM"))
        r_ps = ctx.enter_context(tc.tile_pool(name="rps", bufs=1, space="PSUM"))

        # --- constants, loaded once; scalar HWDGE queue so the sync queue
        # is dedicated to x prefetch ---
        wq_sb = const.tile([P, CT, D], F8)
        nc.scalar.dma_start(wq_sb[:], wq8.rearrange("(t p) d -> p t d", p=P))
        wk_sb = const.tile([P, CT, D], F8)
        wv8_sb = const.tile([P, CT, D], F8)
        wv16_sb = const.tile([P, CT, D], F16)
        bq_sb = const.tile([P, DT], F32)
        nc.scalar.dma_start(bq_sb[:], bq2[:])
        bk_sb = const.tile([P, DT], F32)
        nc.scalar.dma_start(bk_sb[:], bk2[:])
        bv_sb = const.tile([P, D], F32)
        nc.scalar.dma_start(bv_sb[:], bvb[:])
        tri16_sb = const.tile([P, P], F16)
        nc.scalar.dma_start(tri16_sb[:], tri16d[:])
        tri8_sb = const.tile([P, P], F8)
        nc.scalar.dma_start(tri8_sb[:], tri8d[:])
        ones16_sb = const.tile([P, 1], F16)
        nc.vector.memset(ones16_sb[:], WSV)
        ones8_sb = const.tile([P, 2, 1], F8)
        nc.vector.memset(ones8_sb[:], WSV)

        # PE warmup: dummy matmuls with no input deps keep the PE busy while
        # batch-0 inputs stream in (HAM clock-gate to 2.4 GHz).
        warm_sb = const.tile([P, 512], F16)
        nc.vector.memset(warm_sb[:], 0.0)
        for w in range(15):
            wps = proj_ps.tile([P, 512], F32, tag="pp", name=f"warm{w}")
            nc.tensor.matmul(wps[:], warm_sb[:, 0:P], warm_sb[:],
                             start=True, stop=True)

        for b in range(nb):
            # --- X^T tiles: stacked [128, CT, L] e4m3 + [128, CT, 128] f16 ---
            xt8 = x8_pool.tile([P, CT, L], F8, tag="x8", name=f"x8_{b}")
            if b == 0:
                # first batch: l<512 halves of every chunk first so the
                # Q lc=0 groups start earliest
                for ct in range(CT):
                    eng = nc.sync if ct < CT // 2 else nc.gpsimd
                    eng.dma_start(xt8[:, ct, 0:512],
                                  xt8b[b, ct * P:(ct + 1) * P, 0:512])
                for ct in range(CT):
                    nc.sync.dma_start(xt8[:, ct, 512:L],
                                      xt8b[b, ct * P:(ct + 1) * P, 512:L])
            else:
                for ct in range(CT):
                    nc.sync.dma_start(xt8[:, ct, :],
                                      xt8b[b, ct * P:(ct + 1) * P, :])
            xt16 = x16_pool.tile([P, CT, P], F16, tag="x16", name=f"x16_{b}")
            nc.sync.dma_start(xt16[:], xt16b[b].rearrange("(t p) l -> p t l", p=P))
            pm_sb = pm_pool.tile([P, LT], F32, name=f"pm_{b}")
            nc.sync.dma_start(pm_sb[:], pmt[b])
            if b == 0:
                # deferred behind batch-0 x in the sync ring: these don't
                # steal HBM bandwidth from the startup-critical prefetch
                nc.sync.dma_start(wk_sb[:],
                                  wk8.rearrange("(t p) d -> p t d", p=P))
                nc.sync.dma_start(wv16_sb[:],
                                  wv16.rearrange("(t p) d -> p t d", p=P))
                nc.sync.dma_start(wv8_sb[:],
                                  wv8.rearrange("(t p) d -> p t d", p=P))

            # --- Q^T, K^T: fp8 DoubleRow proj; dual-drained carriers:
            # [128d, DT, 128] f16 (corner) + [128d, DT, L] e4m3 (rest) ---
            qt16 = qk_pool.tile([P, DT, P], F16, tag="q16", name=f"q16_{b}")
            qt8 = qk_pool.tile([P, DT, L], F8, tag="q8", name=f"q8_{b}")
            kt16 = qk_pool.tile([P, DT, P], F16, tag="k16", name=f"k16_{b}")
            kt8 = qk_pool.tile([P, DT, L], F8, tag="k8", name=f"k8_{b}")
            for name, w_sb, b_sb, d16, d8 in (
                    ("q", wq_sb, bq_sb, qt16, qt8),
                    ("k", wk_sb, bk_sb, kt16, kt8)):
                if b == 0 and name == "q":
                    # lc-outer so all lc=0 groups run on the early halves
                    order = [(dt, lc) for lc in range(L // 512)
                             for dt in range(DT)]
                else:
                    order = [(dt, lc) for dt in range(DT)
                             for lc in range(L // 512)]
                for dt, lc in order:
                    ps = proj_ps.tile([P, 512], F32, tag="pp",
                                      name=f"{name}ps{dt}_{lc}_{b}")
                    for c in range(CT // 2):
                        nc.tensor.matmul(
                            ps[:],
                            w_sb[:, 2 * c:2 * c + 2, dt * P:(dt + 1) * P],
                            xt8[:, 2 * c:2 * c + 2, lc * 512:(lc + 1) * 512],
                            start=(c == 0), stop=(c == CT // 2 - 1),
                            perf_mode=DR)
                    # (ps + 64b) * 1/64 -> natural-scale carriers
                    if lc == 0:
                        nc.vector.tensor_scalar(
                            d16[:, dt, :], ps[:, 0:P],
                            b_sb[:, dt:dt + 1], 1.0 / WS,
                            mybir.AluOpType.add, mybir.AluOpType.mult)
                        if name == "q":
                            nc.vector.tensor_scalar(
                                d8[:, dt, P:512], ps[:, P:512],
                                b_sb[:, dt:dt + 1], 1.0 / WS,
                                mybir.AluOpType.add, mybir.AluOpType.mult)
                        else:
                            # k8 j<128 is DR-stationary for i>=128 chunks
                            nc.vector.tensor_scalar(
                                d8[:, dt, 0:512], ps[:],
                                b_sb[:, dt:dt + 1], 1.0 / WS,
                                mybir.AluOpType.add, mybir.AluOpType.mult)
                    else:
                        nc.vector.tensor_scalar(
                            d8[:, dt, 512:L], ps[:],
                            b_sb[:, dt:dt + 1], 1.0 / WS,
                            mybir.AluOpType.add, mybir.AluOpType.mult)

            # --- V' = 32*V: j-tile 0 fp16 (dual-drained f16+f8), 1..7 e4m3 ---
            v16 = v_pool.tile([P, D], F16, tag="v16", name=f"v16_{b}")
            v8 = v_pool.tile([P, LT, D], F8, tag="v8", name=f"v8_{b}")
            ps = proj_ps.tile([P, D], F32, tag="pp", name=f"v16ps_{b}")
            for ct in range(CT):
                nc.tensor.matmul(ps[:], xt16[:, ct, :], wv16_sb[:, ct, :],
                                 start=(ct == 0), stop=(ct == CT - 1))
            nc.vector.tensor_add(v16[:], ps[:], bv_sb[:])
            nc.vector.tensor_add(v8[:, 0, :], ps[:], bv_sb[:])
            for lt in range(1, LT):
                ps = proj_ps.tile([P, D], F32, tag="pp", name=f"v8ps{lt}_{b}")
                for c in range(CT // 2):
                    nc.tensor.matmul(
                        ps[:],
                        xt8[:, 2 * c:2 * c + 2, lt * P:(lt + 1) * P],
                        wv8_sb[:, 2 * c:2 * c + 2, :],
                        start=(c == 0), stop=(c == CT // 2 - 1),
                        perf_mode=DR)
                nc.vector.tensor_add(v8[:, lt, :], ps[:], bv_sb[:])

            # --- S^T tiles + exp -> P^T: fp16 corner (i<128), fp8 DR rest ---
            pt16 = pt_pool.tile([P, P], F16, tag="p16", name=f"p16_{b}")
            pt8 = pt_pool.tile([P, LT, L], F8, tag="p8", name=f"p8_{b}")
            for jb in range(LT):
                j0 = jb * P
                if jb == 0:
                    # fp16 corner: rows i<128 see no softmax averaging
                    ps = s_ps.tile([P, P], F32, tag="sp", name=f"spc_{b}")
                    for a in range(DT):
                        nc.tensor.matmul(ps[:], kt16[:, a, :], qt16[:, a, :],
                                         start=(a == 0), stop=(a == DT - 1))
                    nc.scalar.activation(pt16[:], ps[:], AF.Exp,
                                         bias=pm_sb[:, 0:1], scale=SCALE)
                    nc.vector.tensor_mul(pt16[:], pt16[:], tri16_sb[:])
                    i0 = P
                else:
                    i0 = j0
                while i0 < L:
                    n = min((i0 // 512 + 1) * 512, L) - i0
                    ps = s_ps.tile([P, n], F32, tag="sp", name=f"sps{jb}_{i0}_{b}")
                    for a in range(DT // 2):
                        nc.tensor.matmul(
                            ps[:],
                            kt8[:, 2 * a:2 * a + 2, j0:j0 + P],
                            qt8[:, 2 * a:2 * a + 2, i0:i0 + n],
                            start=(a == 0), stop=(a == DT // 2 - 1),
                            perf_mode=DR)
                    nc.scalar.activation(pt8[:, jb, i0:i0 + n], ps[:], AF.Exp,
                                         bias=pm_sb[:, jb:jb + 1], scale=SCALE)
                    i0 += n
                if jb > 0:
                    # mask the diagonal tile: keep j<=i
                    nc.vector.tensor_mul(pt8[:, jb, j0:j0 + P],
                                         pt8[:, jb, j0:j0 + P], tri8_sb[:])

            # --- O' = P^T.T @ V', rowsum' = P^T.T @ 32*ones; norm+relu ---
            for ib in range(LT):
                i0 = ib * P
                ops = o_ps.tile([P, D], F32, tag="op", name=f"ops{ib}_{b}")
                rps = r_ps.tile([P, 1], F32, tag="rp", name=f"rps{ib}_{b}")
                if ib == 0:
                    nc.tensor.matmul(ops[:], pt16[:], v16[:],
                                     start=True, stop=True)
                    nc.tensor.matmul(rps[:], pt16[:], ones16_sb[:],
                                     start=True, stop=True)
                else:
                    js = 0
                    while js <= ib:
                        first = js == 0
                        if js + 1 <= ib:
                            last = js + 2 > ib
                            nc.tensor.matmul(ops[:], pt8[:, js:js + 2, i0:i0 + P],
                                             v8[:, js:js + 2, :],
                                             start=first, stop=last,
                                             perf_mode=DR)
                            nc.tensor.matmul(rps[:], pt8[:, js:js + 2, i0:i0 + P],
                                             ones8_sb[:, 0:2, :],
                                             start=first, stop=last,
                                             perf_mode=DR)
                            js += 2
                        else:
                            nc.tensor.matmul(ops[:], pt8[:, js, i0:i0 + P],
                                             v8[:, js, :],
                                             start=first, stop=True)
                            nc.tensor.matmul(rps[:], pt8[:, js, i0:i0 + P],
                                             ones8_sb[:, 0, 0:1],
                                             start=first, stop=True)
                            js += 1
                rec = sm_pool.tile([P, 1], F32, tag="rec", name=f"rec{ib}_{b}")
                nc.vector.reciprocal(rec[:], rps[:])
                # relu BEFORE the 1/rowsum scaling (rowsum > 0, so
                # relu(O'/rs) == relu(O')/rs): the ACT relu reads the PSUM
                # without waiting on rec, freeing the o_ps slot early
                o_tmp = o_pool.tile([P, D], F16, tag="otmp", name=f"otmp{ib}_{b}")
                nc.scalar.activation(o_tmp[:], ops[:], AF.Relu)
                o_sb = o_pool.tile([P, D], F16, tag="ot", name=f"o{ib}_{b}")
                nc.vector.tensor_scalar_mul(o_sb[:], o_tmp[:], rec[:])
                # SWDGE so stores never head-of-line-block the x prefetch;
                # last batch has no prefetch left -> faster HWDGE
                if b == nb - 1:
                    nc.sync.dma_start(out[b, i0:i0 + P, :], o_sb[:])
                else:
                    nc.gpsimd.dma_start(out[b, i0:i0 + P, :], o_sb[:])

    nc.compile()
    return nc


def _prep_host(x, Wq, bq, Wk, bk, Wv, bv, mask):
    xT = x.transpose(0, 2, 1)  # [B, C, L] view
    xb8 = np.ascontiguousarray(xT).astype(E4NP)
    xb16 = np.ascontiguousarray(xT[:, :, 0:P]).astype(np.float16)
    wq8T = (Wq.T * WS).astype(E4NP)  # [C, D]
    wk8T = (Wk.T * WS).astype(E4NP)
    wv8T = (Wv.T * WSV).astype(E4NP)
    wv16T = (Wv.T * WSV).astype(np.float16)
    bq2 = np.ascontiguousarray(
        (WS * bq).astype(np.float32).reshape(DT, P).T)  # [P, DT]
    bk2 = np.ascontiguousarray((WS * bk).astype(np.float32).reshape(DT, P).T)
    bvb = np.ascontiguousarray(
        np.broadcast_to((WSV * bv).astype(np.float32), (P, D)))  # [P, D]
    pm = np.where(mask[:, 0, :] != 0, 0.0, NEG).astype(np.float32)  # [B, L]
    pmt = np.ascontiguousarray(
        pm.reshape(B, LT, P).transpose(0, 2, 1))  # [B, P, LT]
    tri = (np.arange(P)[:, None] <= np.arange(P)[None, :])
    return (xb8, xb16, wq8T, wk8T, wv8T, wv16T, bq2, bk2, bvb, pmt,
            tri.astype(np.float16), tri.astype(E4NP))


_NC_CACHE = {}


def kernel(x, Wq, bq, Wk, bk, Wv, bv, mask):
    x = np.asarray(x)
    Wq, bq = np.asarray(Wq), np.asarray(bq)
    Wk, bk = np.asarray(Wk), np.asarray(bk)
    Wv, bv = np.asarray(Wv), np.asarray(bv)
    mask = np.asarray(mask)

    (xb8, xb16, wq8T, wk8T, wv8T, wv16T, bq2, bk2, bvb, pmt,
     tri16, tri8) = _prep_host(x, Wq, bq, Wk, bk, Wv, bv, mask)

    if "nc" not in _NC_CACHE:
        _NC_CACHE["nc"] = build_program(NB)
    nc = _NC_CACHE["nc"]

    in_maps = []
    for c in range(N_CORES):
        s = slice(c * NB, (c + 1) * NB)
        in_maps.append({
            "xt8b": np.ascontiguousarray(xb8[s]),
            "xt16b": np.ascontiguousarray(xb16[s]),
            "wq8": wq8T, "wk8": wk8T, "wv8": wv8T, "wv16": wv16T,
            "bq2": bq2, "bk2": bk2, "bvb": bvb,
            "pmt": np.ascontiguousarray(pmt[s]),
            "tri16d": tri16, "tri8d": tri8,
        })

    res = bass_utils.run_bass_kernel_spmd(
        nc, in_maps, core_ids=list(range(N_CORES)),
        trace=bool(int(os.environ.get("KERNEL_TRACE", "0"))),
    )
    if os.environ.get("KERNEL_RESULT_HOOK"):
        _NC_CACHE["last_result"] = res

    return np.concatenate(
        [res.results[c]["out"].astype(np.float32) for c in range(N_CORES)],
        axis=0)
